# revision 19
# baseline (speedup 1.0000x reference)
"""Trainium2 Bass kernel for nn_Meta_67078799229377 (relation-network meta-learner).

Sharding: 8 cores = 4 batch elements x 2 halves of the relation-j axis.
Each core runs the full backbone for its batch element's 6 images, then the
relation network for its 18 (i, j) pairs, fully fused on-chip (the
[s,s,m,m,128] tensor never exists in HBM).

v2 layout:
  - All constants packed into two HBM tensors (bf16 + f32) -> 4 input DMAs.
  - PE warm-up matmuls at t=0 so the tensor engine is at full p-state when
    conv1 starts.
  - ACT runs Relu only (no activation-table reloads); cls softmax and the
    score-head MLP/sigmoid/losses run on the host from raw logits and the
    128x18 relation sums.
  - relation stage: hdd elementwise generation split DVE/ACT/Pool (tunable),
    g = relu(.)+sum split ACT/Pool via accum_out.
"""
import os
import numpy as np
import ml_dtypes

import concourse.bass as bass
import concourse.mybir as mybir
import concourse.tile as tile
from concourse import bacc
from concourse.bass_utils import run_bass_kernel_spmd

F32 = mybir.dt.float32
BF16 = mybir.dt.bfloat16
AF = mybir.ActivationFunctionType
OP = mybir.AluOpType

B, S, D = 4, 6, 8
M = D * D            # 64 spatial positions
C2 = 66              # 64 channels + 2 coord channels
H1 = 128             # g-MLP hidden
CO = 64              # g-MLP out
NCls = 64
N_CORES = 8

# ---- packed-constant column offsets (bf16 tensor) ----
O_W1 = 0              # [27, 32]
O_W2 = 32             # [96, 3*48] (dy-stacked conv2 weights)
O_W3 = O_W2 + 144     # [48, 9*64]
O_W1A = O_W3 + 576    # [66, 128]
O_W1B = O_W1A + 128   # [66, 128]
O_WG2 = O_W1B + 128   # [128, 64]
NBF = O_WG2 + 64

# ---- packed-constant column offsets (f32 tensor) ----
OF_BC1 = 0            # [32, 1]
OF_BC2 = 1            # [48, 1]
OF_BC3 = 2            # [64, 1]
OF_BG1 = 3            # [128, 1]
OF_BG2 = 4            # [128, 1] (bg2 duplicated x2)
OF_WLE = 5            # [65, 64]
OF_BC1Q = OF_WLE + 64 # [128, 1] bc1 tiled x4 for the packed conv1 relu
NF = OF_BC1Q + 1

# hdd-gen engine assignment per local q (0..31 within a unit); rest on DVE.
# GPSIMD (Pool) cannot touch PSUM, so it only gets SBUF-side hdd work and the
# g-relu/sum instructions are split ACT/DVE.
HDD_ACT_Q = frozenset((15, 31))
HDD_POOL_Q = frozenset()
# g-relu+sum runs on ACT only — DVE accum_out is broken on HW (wrong results,
# 70us/instr) and GPSIMD tensor_scalar runs at ~15ns/row.
# relation blocks: (jl, q_base, n_q); final unit split for a faster drain.
BLOCKS = ((0, 0, 32), (0, 32, 32), (1, 0, 32), (1, 32, 32),
          (2, 0, 32), (2, 32, 16), (2, 48, 16))
NXF = 3 * len(BLOCKS)
N_WARMUP_MM = 5


def _build_nc():
    nc = bacc.Bacc("TRN2", target_bir_lowering=False, debug=False,
                   num_devices=N_CORES)

    x_patches = nc.dram_tensor("patches", [27, S * 1024], BF16, kind="ExternalInput")
    x_cb = nc.dram_tensor("cb", [128, NBF], BF16, kind="ExternalInput")
    x_cf = nc.dram_tensor("cf", [128, NF], F32, kind="ExternalInput")
    x_coords = nc.dram_tensor("coords", [2, S * M], BF16, kind="ExternalInput")

    out_xf = nc.dram_tensor("xf", [128, NXF], F32, kind="ExternalOutput")
    out_logits = nc.dram_tensor("logits", [S, NCls], F32, kind="ExternalOutput")

    with tile.TileContext(nc) as tc:
        with (
            tc.tile_pool(name="const", bufs=1) as cpool,
            tc.tile_pool(name="work", bufs=1) as wpool,
            tc.tile_pool(name="hdd", bufs=2) as hpool,
        ):
            # ---- input DMAs: spread across engine queues so descriptor
            # generation runs in parallel (the SP sequencer serializes at
            # ~700ns per dma_start) ----
            cb = cpool.tile([128, NBF], BF16)
            patches_sb = cpool.tile([27, S * 1024], BF16)
            cf = cpool.tile([128, NF], F32)
            featc = wpool.tile([C2, S * M], BF16)
            nc.sync.dma_start(out=cb[:, 0:O_W1A], in_=x_cb[:, 0:O_W1A])
            nc.sync.dma_start(out=patches_sb[:, 0:2048], in_=x_patches[:, 0:2048])
            nc.scalar.dma_start(out=patches_sb[:, 2048:4096],
                                in_=x_patches[:, 2048:4096])
            nc.sync.dma_start(out=patches_sb[:, 4096:6144],
                              in_=x_patches[:, 4096:6144])
            nc.scalar.dma_start(out=cf[:], in_=x_cf[:])


            w1 = cb[0:27, O_W1:O_W1 + 32]
            w2 = cb[0:96, O_W2:O_W2 + 144]
            w3 = cb[0:48, O_W3:O_W3 + 576]
            w1a = cb[0:C2, O_W1A:O_W1A + 128]
            w1b = cb[0:C2, O_W1B:O_W1B + 128]
            wg2 = cb[0:H1, O_WG2:O_WG2 + 64]
            bc1 = cf[0:32, OF_BC1:OF_BC1 + 1]
            bc2 = cf[0:48, OF_BC2:OF_BC2 + 1]
            bc3 = cf[0:64, OF_BC3:OF_BC3 + 1]
            bg1 = cf[0:H1, OF_BG1:OF_BG1 + 1]
            bg2 = cf[0:128, OF_BG2:OF_BG2 + 1]
            wle = cf[0:65, OF_WLE:OF_WLE + 64]
            bc1q = cf[0:128, OF_BC1Q:OF_BC1Q + 1]

            # conv1/conv2 outputs stored with even/odd columns split so the
            # stride-2 conv taps read packed columns (PE streams at 1 col/cyc
            # only when the moving AP's innermost dim is contiguous).
            # conv1 output, quad-split: even/odd cols and rows in separate
            # planes so the dy-stack DMA below moves contiguous blocks, and
            # 4 images stacked on the partition dim (partition = lane*32+ch)
            # so each relu instruction uses the full engine width.
            c1p4 = wpool.tile([128, 2, 2, 2, 17, 17], BF16)
            # conv2 input: dy-stacked (96 = 3dy x 32ch) + even/odd columns, so
            # conv2 needs only 3 accumulating taps of contraction 96.
            c1s = wpool.tile([96, S, 2, 16, 17], BF16)
            c2p = wpool.tile([48, S, 2, 17, 9], BF16)
            xf_cols = wpool.tile([2 * CO, NXF], F32)
            u_f32 = wpool.tile([H1, 3 * M], F32)
            v_bf = wpool.tile([H1, S * M], BF16)
            fme = wpool.tile([65, S], F32)
            logits_sb = wpool.tile([S, NCls], F32)
            wu = wpool.tile([128, 512], BF16)

            # warm-up source first (the PE warm-up waits on it), then the
            # remaining input DMAs, then only the conv PAD strips (the relus
            # overwrite everything else), all on Pool which is otherwise idle.
            nc.gpsimd.memset(wu[:], 0.0)
            nc.gpsimd.dma_start(out=featc[64:66, :], in_=x_coords[:])
            nc.gpsimd.dma_start(out=cb[:, O_W1A:NBF], in_=x_cb[:, O_W1A:NBF])
            nc.gpsimd.memset(fme[64:65, :], 1.0)
            nc.gpsimd.memset(c1p4[:, :, 0, 0, 16, :], 0.0)  # input row 32
            nc.gpsimd.memset(c1p4[:, :, 0, :, :, 16], 0.0)  # input col 32
            nc.gpsimd.memset(c1p4[:, :, 1, 0, 16, :], 0.0)
            nc.gpsimd.memset(c1p4[:, :, 1, :, :, 16], 0.0)
            nc.gpsimd.memset(c2p[:, :, :, 16, :], 0.0)      # input row 16
            nc.gpsimd.memset(c2p[:, :, 0, :, 8], 0.0)       # input col 16
            # preload the ACT function table during the DMA wait
            nc.scalar.activation(logits_sb[0:1, 0:1], fme[64:65, 0:1], AF.Relu)

            # ================= conv + cls + uv scope =================
            with (
                tc.tile_pool(name="pc1", bufs=2, space="PSUM") as p1_pool,
                tc.tile_pool(name="pc2", bufs=2, space="PSUM") as p2_pool,
                tc.tile_pool(name="psm", bufs=2, space="PSUM") as ps_pool,
            ):
                # PE warm-up: ramp the p-state while DMAs land.
                wu_ps = ps_pool.tile([128, 512], F32, tag="sm")
                for _ in range(N_WARMUP_MM):
                    nc.tensor.matmul(wu_ps[:], wu[:, 0:128], wu[:],
                                     start=True, stop=True)

                # ---- conv1: [27]->[32], 64x64 -> 32x32 (im2col'd); 4
                # images go to 4 PE column-tiles so the relus run full-width
                for g in range(2):
                    imgs = list(range(4 * g, min(4 * g + 4, S)))
                    n = len(imgs)
                    ps1 = p1_pool.tile([32 * n, 2, 16, 32], F32, tag="c1")
                    for k, img in enumerate(imgs):
                        for h in range(2):
                            nc.tensor.matmul(
                                ps1[32 * k:32 * k + 32, h, :, :].rearrange(
                                    "p a b -> p (a b)"),
                                w1,
                                patches_sb[:, img * 1024 + h * 512:
                                           img * 1024 + (h + 1) * 512],
                                start=True, stop=True,
                                tile_position=(0, 32 * k))
                    for cp in range(2):
                        for rp in range(2):
                            out_ap = c1p4[0:32 * n, g, cp, rp, 0:16,
                                          0:16].rearrange(
                                "p (h r) c -> p h r c", h=2)
                            in_ap = ps1[:, :, rp:16:2, cp:32:2]
                            bq = bc1q[0:32 * n, :]
                            if (g * 2 + cp + rp) % 2 == 0:
                                nc.vector.tensor_scalar(out_ap, in_ap, bq,
                                                        0.0, op0=OP.add,
                                                        op1=OP.max)
                            else:
                                nc.scalar.activation(out_ap, in_ap, AF.Relu,
                                                     bias=bq)

                # ---- dy-stack c1 into 96 partitions (DMA engines are idle) ----
                for img in range(S):
                    g, k = divmod(img, 4)
                    for dy in range(3):
                        eng = (nc.sync, nc.scalar, nc.gpsimd)[(img + dy) % 3]
                        rp, rb = dy % 2, dy // 2
                        eng.dma_start(
                            out=c1s[32 * dy:32 * dy + 32, img, :, :,
                                    :].rearrange("p cp r c -> p cp (r c)"),
                            in_=c1p4[32 * k:32 * k + 32, g, :, rp,
                                     rb:rb + 16, :].rearrange(
                                "p cp r c -> p cp (r c)"))

                # ---- conv2: [96]->[48] over 3 dx taps, 32x32 -> 16x16 ----
                for ip in range(3):      # image pairs
                    ps2 = p2_pool.tile([48, 2, 16, 16], F32, tag="c2")
                    for dx in range(3):
                        nc.tensor.matmul(
                            ps2[:],
                            w2[:, dx * 48:(dx + 1) * 48],
                            c1s[:, 2 * ip:2 * ip + 2, dx % 2,
                                :, dx // 2:dx // 2 + 16],
                            start=(dx == 0), stop=(dx == 2))
                    for par in range(2):
                        out_ap = c2p[:, 2 * ip:2 * ip + 2, par, 0:16, 0:8]
                        in_ap = ps2[:, :, :, par:16:2]
                        if (ip * 2 + par) % 2 == 0:
                            nc.vector.tensor_scalar(out_ap, in_ap, bc2, 0.0,
                                                    op0=OP.add, op1=OP.max)
                        else:
                            nc.scalar.activation(out_ap, in_ap, AF.Relu,
                                                 bias=bc2)

                # ---- conv3: [48]->[64], 16x16 -> 8x8 ----
                ps3 = ps_pool.tile([64, S, D, D], F32, tag="sm")
                for k, (dy, dx) in enumerate(
                        (dy, dx) for dy in range(3) for dx in range(3)):
                    nc.tensor.matmul(
                        ps3[:],
                        w3[:, k * 64:(k + 1) * 64],
                        c2p[:, :, dx % 2, dy:dy + 15:2, dx // 2:dx // 2 + 8],
                        start=(k == 0), stop=(k == 8))
                nc.scalar.activation(featc[0:64, :].rearrange("p (i m) -> p i m", m=M),
                                     ps3[:].rearrange("p i a b -> p i (a b)"),
                                     AF.Relu, bias=bc3)

                # ---- cls head: raw logits only ----
                nc.vector.tensor_reduce(
                    fme[0:64, :], featc[0:64, :].rearrange("p (i m) -> p i m", m=M),
                    axis=mybir.AxisListType.X, op=OP.add)
                psl = ps_pool.tile([S, NCls], F32, tag="sm")
                nc.tensor.matmul(psl[:], fme[:], wle, start=True, stop=True)
                nc.vector.tensor_copy(logits_sb[:], psl[:])
                nc.sync.dma_start(out=out_logits[:], in_=logits_sb[:])

                # ---- u / v ----
                psu = ps_pool.tile([H1, S * M], F32, tag="sm")
                psv = ps_pool.tile([H1, S * M], F32, tag="sm")
                nc.tensor.matmul(psu[:], w1a, featc[:], start=True, stop=True)
                nc.tensor.matmul(psv[:], w1b, featc[:], start=True, stop=True)
                nc.vector.tensor_copy(u_f32[:], psu[:, 0:3 * M])
                nc.vector.tensor_scalar(v_bf[:], psv[:], bg1, None, op0=OP.add)

            # ================= relation scope =================
            with tc.tile_pool(name="pbig", bufs=2, space="PSUM") as pb_pool:
                for bi, (jl, qb, nq) in enumerate(BLOCKS):
                    hdd = hpool.tile([H1, nq, S * M], BF16, tag="hdd")
                    for ql in range(nq):
                        q = qb + ql
                        ucol = u_f32[:, jl * M + q: jl * M + q + 1]
                        if (ql % 32) in HDD_ACT_Q:
                            nc.scalar.activation(hdd[:, ql, :], v_bf[:],
                                                 AF.Relu, bias=ucol)
                        else:
                            nc.vector.tensor_scalar(hdd[:, ql, :], v_bf[:],
                                                    ucol, 0.0,
                                                    op0=OP.add, op1=OP.max)
                    for duo in range(3):
                        iA, iB = 2 * duo, 2 * duo + 1
                        ps = pb_pool.tile([2 * CO, nq * 64], F32, tag="gps")
                        for qg in range(nq // 8):
                            nc.tensor.matmul(
                                ps[0:CO, qg * 512:(qg + 1) * 512],
                                wg2,
                                hdd[:, qg * 8:(qg + 1) * 8, iA * M:(iA + 1) * M],
                                start=True, stop=True)
                            nc.tensor.matmul(
                                ps[CO:2 * CO, qg * 512:(qg + 1) * 512],
                                wg2,
                                hdd[:, qg * 8:(qg + 1) * 8, iB * M:(iB + 1) * M],
                                start=True, stop=True,
                                tile_position=(0, 64))
                        col = 3 * bi + duo
                        # relu in place over the PSUM tile: PSUM access is
                        # faster for ACT than SBUF and it avoids 4KB/partition
                        # of SBUF writes contending with the DVE hdd stream
                        nc.scalar.activation(
                            ps[:], ps[:], AF.Relu, bias=bg2,
                            accum_out=xf_cols[:, col:col + 1])

            nc.sync.dma_start(out=out_xf[:], in_=xf_cols[:])
    nc.compile()
    return nc


_NC_CACHE = None


def _get_nc():
    global _NC_CACHE
    if _NC_CACHE is None:
        _NC_CACHE = _build_nc()
    return _NC_CACHE


def _host_prep(inputs):
    ins = {k: np.asarray(v) for k, v in inputs.items()}
    x = np.concatenate([ins['support_x'], ins['query_x']], axis=1)
    lab = np.concatenate([ins['support_y'], ins['query_y']], axis=1)

    xpad = np.pad(x.astype(np.float32), ((0, 0), (0, 0), (0, 0), (0, 1), (0, 1)))
    win = np.lib.stride_tricks.sliding_window_view(xpad, (3, 3), axis=(3, 4))
    win = win[:, :, :, ::2, ::2]
    patches = win.transpose(0, 2, 5, 6, 1, 3, 4).reshape(B, 27, S, 1024)
    patches = np.ascontiguousarray(patches, np.float32)

    f32 = np.float32
    bf16 = ml_dtypes.bfloat16

    cbf = np.zeros((128, NBF), f32)
    cbf[0:27, O_W1:O_W1 + 32] = ins['k1'].reshape(32, 27).T
    cbf[0:96, O_W2:O_W2 + 144] = ins['k2'].transpose(2, 1, 3, 0).reshape(96, 144)
    cbf[0:48, O_W3:O_W3 + 576] = ins['k3'].transpose(1, 2, 3, 0).reshape(48, 576)
    Wg1 = ins['Wg1'].astype(f32)
    cbf[0:C2, O_W1A:O_W1A + 128] = Wg1[:C2]
    cbf[0:C2, O_W1B:O_W1B + 128] = Wg1[C2:]
    cbf[0:H1, O_WG2:O_WG2 + 64] = ins['Wg2']
    cbf = cbf.astype(bf16)

    cff = np.zeros((128, NF), f32)
    cff[0:32, OF_BC1] = ins['bc1']
    cff[0:48, OF_BC2] = ins['bc2']
    cff[0:64, OF_BC3] = ins['bc3']
    cff[0:H1, OF_BG1] = ins['bg1']
    cff[0:128, OF_BG2] = np.tile(ins['bg2'].astype(f32), 2)
    cff[0:65, OF_WLE:OF_WLE + 64] = np.vstack(
        [ins['Wlog'].astype(f32) / M, ins['blog'][None, :].astype(f32)])
    cff[0:128, OF_BC1Q] = np.tile(ins['bc1'].astype(f32), 4)

    ii = np.arange(D, dtype=f32) / D
    coord = np.stack([np.broadcast_to(ii[:, None], (D, D)),
                      np.broadcast_to(ii[None, :], (D, D))]).reshape(2, M)
    coords = np.ascontiguousarray(np.tile(coord, (1, S)), f32).astype(bf16)

    common = dict(cb=cbf, cf=np.ascontiguousarray(cff), coords=coords)
    in_maps = []
    for core in range(N_CORES):
        b, half = core // 2, core % 2
        # odd cores see images in rotated order so the program's local
        # j in {0,1,2} maps to global j in {3,4,5}
        perm = (0, 1, 2, 3, 4, 5) if half == 0 else (3, 4, 5, 0, 1, 2)
        m = dict(common)
        m['patches'] = np.ascontiguousarray(
            patches[b][:, perm, :]).reshape(27, S * 1024).astype(bf16)
        in_maps.append(m)
    return in_maps, lab, ins


def _host_post(results, lab, ins):
    f32 = np.float32
    # ---- cls loss from raw logits (even cores have identity perm) ----
    cls_terms = np.zeros((B, S), f32)
    for b in range(B):
        logits = results[2 * b]["logits"].astype(f32)          # [6, 64]
        mx = logits.max(axis=1, keepdims=True)
        lse = np.log(np.exp(logits - mx).sum(axis=1, keepdims=True)) + mx
        logp = logits - lse
        cls_terms[b] = lse[:, 0] - logits[np.arange(S), lab[b]]
    cls_loss = np.float32(cls_terms.mean())

    # ---- score head from relation sums ----
    xf = np.zeros((B, S, S, 2 * CO), f32)   # [b, i_loc?, ...]
    P = np.zeros((B, S, S), f32)
    Wf1, bf1 = ins['Wf1'].astype(f32), ins['bf1'].astype(f32)
    Wf2, bf2 = ins['Wf2'].astype(f32), ins['bf2'].astype(f32)
    for core in range(N_CORES):
        b, half = core // 2, core % 2
        perm = (0, 1, 2, 3, 4, 5) if half == 0 else (3, 4, 5, 0, 1, 2)
        dev = results[core]["xf"].astype(f32)                  # [128, NXF]
        for jl in range(3):
            for duo in range(3):
                colsum = np.zeros(128, f32)
                for bi, (bjl, _, _) in enumerate(BLOCKS):
                    if bjl == jl:
                        colsum += dev[:, 3 * bi + duo]
                for ih in range(2):
                    i_loc = 2 * duo + ih
                    x_f = colsum[ih * CO:(ih + 1) * CO]
                    h = np.maximum(x_f @ Wf1 + bf1, 0.0)
                    s2 = h @ Wf2 + bf2
                    P[b, perm[i_loc], perm[jl]] = 1.0 / (1.0 + np.exp(-s2[0]))

    y = (lab[:, :, None] == lab[:, None, :]).astype(f32)
    Pt = P.transpose(0, 2, 1)
    sym, anti = f32(0.5) * (P + Pt), f32(0.5) * (P - Pt)
    sym_n = np.sqrt((sym ** 2).sum(axis=(1, 2)))
    anti_n = np.sqrt((anti ** 2).sum(axis=(1, 2)))
    sym_loss = np.float32(((sym_n - anti_n) / (sym_n + anti_n)).mean())
    euc_loss = np.float32(((P - y) ** 2).mean())
    rn_loss = np.float32(euc_loss - np.float32(0.1) * sym_loss)
    return np.float32(cls_loss), np.float32(rn_loss), np.float32(sym_loss)


def run_spmd(inputs, trace=False, **kwargs):
    nc = _get_nc()
    in_maps, lab, ins = _host_prep(inputs)
    res = run_bass_kernel_spmd(nc, in_maps, list(range(N_CORES)),
                               trace=trace, **kwargs)
    return _host_post(res.results, lab, ins), res


def kernel(**inputs):
    out, _ = run_spmd(inputs)
    return out


# revision 20
# speedup vs baseline: 1.1832x; 1.1832x over previous
"""Trainium2 Bass kernel for nn_Meta_67078799229377 (relation-network meta-learner).

Sharding: 8 cores = 4 batch elements x 2 halves of the relation-j axis.
Each core runs the full backbone for its batch element's 6 images, then the
relation network for its 18 (i, j) pairs, fully fused on-chip (the
[s,s,m,m,128] tensor never exists in HBM).

v2 layout:
  - All constants packed into two HBM tensors (bf16 + f32) -> 4 input DMAs.
  - PE warm-up matmuls at t=0 so the tensor engine is at full p-state when
    conv1 starts.
  - ACT runs Relu only (no activation-table reloads); cls softmax and the
    score-head MLP/sigmoid/losses run on the host from raw logits and the
    128x18 relation sums.
  - relation stage: hdd elementwise generation split DVE/ACT/Pool (tunable),
    g = relu(.)+sum split ACT/Pool via accum_out.
"""
import os
import numpy as np
import ml_dtypes

import concourse.bass as bass
import concourse.mybir as mybir
import concourse.tile as tile
from concourse import bacc
from concourse.bass_utils import run_bass_kernel_spmd

F32 = mybir.dt.float32
BF16 = mybir.dt.bfloat16
AF = mybir.ActivationFunctionType
OP = mybir.AluOpType

B, S, D = 4, 6, 8
M = D * D            # 64 spatial positions
C2 = 66              # 64 channels + 2 coord channels
H1 = 128             # g-MLP hidden
CO = 64              # g-MLP out
NCls = 64
N_CORES = 8

# ---- packed-constant column offsets (bf16 tensor) ----
O_W1 = 0              # [27, 32]
O_W2 = 32             # [96, 3*48] (dy-stacked conv2 weights)
O_W3 = O_W2 + 144     # [48, 9*64]
O_W1A = O_W3 + 576    # [66, 128]
O_W1B = O_W1A + 128   # [66, 128]
O_WG2 = O_W1B + 128   # [128, 64]
NBF = O_WG2 + 64

# ---- packed-constant column offsets (f32 tensor) ----
OF_BC1 = 0            # [32, 1]
OF_BC2 = 1            # [48, 1]
OF_BC3 = 2            # [64, 1]
OF_BG1 = 3            # [128, 1]
OF_BG2 = 4            # [128, 1] (bg2 duplicated x2)
OF_WLE = 5            # [65, 64]
OF_BC1Q = OF_WLE + 64 # [128, 1] bc1 tiled x4 for the packed conv1 relu
NF = OF_BC1Q + 1

# hdd-gen engine assignment per local q (0..31 within a unit); rest on DVE.
# GPSIMD (Pool) cannot touch PSUM, so it only gets SBUF-side hdd work and the
# g-relu/sum instructions are split ACT/DVE.
HDD_ACT_Q = frozenset((15, 31))
HDD_POOL_Q = frozenset()
# g-relu+sum runs on ACT only — DVE accum_out is broken on HW (wrong results,
# 70us/instr) and GPSIMD tensor_scalar runs at ~15ns/row.
# relation blocks: (jl, q_base, n_q); final unit split for a faster drain.
BLOCKS = ((0, 0, 32), (0, 32, 32), (1, 0, 32), (1, 32, 32),
          (2, 0, 32), (2, 32, 16), (2, 48, 16))
NXF = 3 * len(BLOCKS)
N_WARMUP_MM = 7


def _build_nc():
    nc = bacc.Bacc("TRN2", target_bir_lowering=False, debug=False,
                   num_devices=N_CORES)

    x_patches = nc.dram_tensor("patches", [27, S * 1024], BF16, kind="ExternalInput")
    x_cb = nc.dram_tensor("cb", [128, NBF], BF16, kind="ExternalInput")
    x_cf = nc.dram_tensor("cf", [128, NF], F32, kind="ExternalInput")
    x_coords = nc.dram_tensor("coords", [2, S * M], BF16, kind="ExternalInput")

    out_xf = nc.dram_tensor("xf", [128, NXF], F32, kind="ExternalOutput")
    out_logits = nc.dram_tensor("logits", [S, NCls], F32, kind="ExternalOutput")

    with tile.TileContext(nc) as tc:
        with (
            tc.tile_pool(name="const", bufs=1) as cpool,
            tc.tile_pool(name="work", bufs=1) as wpool,
            tc.tile_pool(name="hdd", bufs=2) as hpool,
            tc.tile_pool(name="gscr", bufs=2) as spool,
        ):
            # ---- input DMAs: spread across engine queues so descriptor
            # generation runs in parallel (the SP sequencer serializes at
            # ~700ns per dma_start) ----
            cb = cpool.tile([128, NBF], BF16)
            patches_sb = cpool.tile([27, S * 1024], BF16)
            cf = cpool.tile([128, NF], F32)
            featc = wpool.tile([C2, S * M], BF16)
            nc.sync.dma_start(out=cb[:, 0:O_W1A], in_=x_cb[:, 0:O_W1A])
            nc.sync.dma_start(out=patches_sb[:, 0:2048], in_=x_patches[:, 0:2048])
            nc.scalar.dma_start(out=patches_sb[:, 2048:4096],
                                in_=x_patches[:, 2048:4096])
            nc.sync.dma_start(out=patches_sb[:, 4096:6144],
                              in_=x_patches[:, 4096:6144])
            nc.scalar.dma_start(out=cf[:], in_=x_cf[:])


            w1 = cb[0:27, O_W1:O_W1 + 32]
            w2 = cb[0:96, O_W2:O_W2 + 144]
            w3 = cb[0:48, O_W3:O_W3 + 576]
            w1a = cb[0:C2, O_W1A:O_W1A + 128]
            w1b = cb[0:C2, O_W1B:O_W1B + 128]
            wg2 = cb[0:H1, O_WG2:O_WG2 + 64]
            bc1 = cf[0:32, OF_BC1:OF_BC1 + 1]
            bc2 = cf[0:48, OF_BC2:OF_BC2 + 1]
            bc3 = cf[0:64, OF_BC3:OF_BC3 + 1]
            bg1 = cf[0:H1, OF_BG1:OF_BG1 + 1]
            bg2 = cf[0:128, OF_BG2:OF_BG2 + 1]
            wle = cf[0:65, OF_WLE:OF_WLE + 64]
            bc1q = cf[0:128, OF_BC1Q:OF_BC1Q + 1]

            # conv1/conv2 outputs stored with even/odd columns split so the
            # stride-2 conv taps read packed columns (PE streams at 1 col/cyc
            # only when the moving AP's innermost dim is contiguous).
            # conv1 output, quad-split: even/odd cols and rows in separate
            # planes so the dy-stack DMA below moves contiguous blocks, and
            # 4 images stacked on the partition dim (partition = lane*32+ch)
            # so each relu instruction uses the full engine width.
            c1p4 = wpool.tile([128, 2, 2, 2, 17, 17], BF16)
            # conv2 input: dy-stacked (96 = 3dy x 32ch) + even/odd columns, so
            # conv2 needs only 3 accumulating taps of contraction 96.
            c1s = wpool.tile([96, S, 2, 16, 17], BF16)
            c2p = wpool.tile([48, S, 2, 17, 9], BF16)
            xf_cols = wpool.tile([2 * CO, NXF], F32)
            u_f32 = wpool.tile([H1, 3 * M], F32)
            v_bf = wpool.tile([H1, S * M], BF16)
            fme = wpool.tile([65, S], F32)
            logits_sb = wpool.tile([S, NCls], F32)
            wu = wpool.tile([128, 512], BF16)

            # warm-up source first (the PE warm-up waits on it), then the
            # remaining input DMAs, then only the conv PAD strips (the relus
            # overwrite everything else), all on Pool which is otherwise idle.
            nc.gpsimd.memset(wu[:], 0.0)
            nc.gpsimd.dma_start(out=featc[64:66, :], in_=x_coords[:])
            nc.gpsimd.dma_start(out=cb[:, O_W1A:NBF], in_=x_cb[:, O_W1A:NBF])
            nc.gpsimd.memset(fme[64:65, :], 1.0)
            nc.gpsimd.memset(c1p4[:, :, 0, 0, 16, :], 0.0)  # input row 32
            nc.gpsimd.memset(c1p4[:, :, 0, :, :, 16], 0.0)  # input col 32
            nc.gpsimd.memset(c1p4[:, :, 1, 0, 16, :], 0.0)
            nc.gpsimd.memset(c1p4[:, :, 1, :, :, 16], 0.0)
            nc.gpsimd.memset(c2p[:, :, :, 16, :], 0.0)      # input row 16
            nc.gpsimd.memset(c2p[:, :, 0, :, 8], 0.0)       # input col 16
            # preload the ACT function table during the DMA wait
            nc.scalar.activation(logits_sb[0:1, 0:1], fme[64:65, 0:1], AF.Relu)

            # ================= conv + cls + uv scope =================
            with (
                tc.tile_pool(name="pc1", bufs=2, space="PSUM") as p1_pool,
                tc.tile_pool(name="pc2", bufs=2, space="PSUM") as p2_pool,
                tc.tile_pool(name="psm", bufs=2, space="PSUM") as ps_pool,
            ):
                # PE warm-up: ramp the p-state while DMAs land.
                wu_ps = ps_pool.tile([128, 512], F32, tag="sm")
                for _ in range(N_WARMUP_MM):
                    nc.tensor.matmul(wu_ps[:], wu[:, 0:128], wu[:],
                                     start=True, stop=True)

                # ---- conv1: [27]->[32], 64x64 -> 32x32 (im2col'd); 4
                # images go to 4 PE column-tiles so the relus run full-width
                for g in range(2):
                    imgs = list(range(4 * g, min(4 * g + 4, S)))
                    n = len(imgs)
                    ps1 = p1_pool.tile([32 * n, 2, 16, 32], F32, tag="c1")
                    for k, img in enumerate(imgs):
                        for h in range(2):
                            nc.tensor.matmul(
                                ps1[32 * k:32 * k + 32, h, :, :].rearrange(
                                    "p a b -> p (a b)"),
                                w1,
                                patches_sb[:, img * 1024 + h * 512:
                                           img * 1024 + (h + 1) * 512],
                                start=True, stop=True,
                                tile_position=(0, 32 * k))
                    for cp in range(2):
                        for rp in range(2):
                            out_ap = c1p4[0:32 * n, g, cp, rp, 0:16,
                                          0:16].rearrange(
                                "p (h r) c -> p h r c", h=2)
                            in_ap = ps1[:, :, rp:16:2, cp:32:2]
                            bq = bc1q[0:32 * n, :]
                            if (g * 2 + cp + rp) % 2 == 0:
                                nc.vector.tensor_scalar(out_ap, in_ap, bq,
                                                        0.0, op0=OP.add,
                                                        op1=OP.max)
                            else:
                                nc.scalar.activation(out_ap, in_ap, AF.Relu,
                                                     bias=bq)

                # ---- dy-stack c1 into 96 partitions (DMA engines are idle) ----
                for img in range(S):
                    g, k = divmod(img, 4)
                    for dy in range(3):
                        eng = (nc.sync, nc.scalar, nc.gpsimd)[(img + dy) % 3]
                        rp, rb = dy % 2, dy // 2
                        eng.dma_start(
                            out=c1s[32 * dy:32 * dy + 32, img, :, :,
                                    :].rearrange("p cp r c -> p cp (r c)"),
                            in_=c1p4[32 * k:32 * k + 32, g, :, rp,
                                     rb:rb + 16, :].rearrange(
                                "p cp r c -> p cp (r c)"))

                # ---- conv2: [96]->[48] over 3 dx taps, 32x32 -> 16x16 ----
                for ip in range(3):      # image pairs
                    ps2 = p2_pool.tile([48, 2, 16, 16], F32, tag="c2")
                    for dx in range(3):
                        nc.tensor.matmul(
                            ps2[:],
                            w2[:, dx * 48:(dx + 1) * 48],
                            c1s[:, 2 * ip:2 * ip + 2, dx % 2,
                                :, dx // 2:dx // 2 + 16],
                            start=(dx == 0), stop=(dx == 2))
                    for par in range(2):
                        out_ap = c2p[:, 2 * ip:2 * ip + 2, par, 0:16, 0:8]
                        in_ap = ps2[:, :, :, par:16:2]
                        if (ip * 2 + par) % 2 == 0:
                            nc.vector.tensor_scalar(out_ap, in_ap, bc2, 0.0,
                                                    op0=OP.add, op1=OP.max)
                        else:
                            nc.scalar.activation(out_ap, in_ap, AF.Relu,
                                                 bias=bc2)

                # ---- conv3: [48]->[64], 16x16 -> 8x8 ----
                ps3 = ps_pool.tile([64, S, D, D], F32, tag="sm")
                for k, (dy, dx) in enumerate(
                        (dy, dx) for dy in range(3) for dx in range(3)):
                    nc.tensor.matmul(
                        ps3[:],
                        w3[:, k * 64:(k + 1) * 64],
                        c2p[:, :, dx % 2, dy:dy + 15:2, dx // 2:dx // 2 + 8],
                        start=(k == 0), stop=(k == 8))
                nc.scalar.activation(featc[0:64, :].rearrange("p (i m) -> p i m", m=M),
                                     ps3[:].rearrange("p i a b -> p i (a b)"),
                                     AF.Relu, bias=bc3)

                # ---- cls head: raw logits only ----
                nc.vector.tensor_reduce(
                    fme[0:64, :], featc[0:64, :].rearrange("p (i m) -> p i m", m=M),
                    axis=mybir.AxisListType.X, op=OP.add)
                psl = ps_pool.tile([S, NCls], F32, tag="sm")
                nc.tensor.matmul(psl[:], fme[:], wle, start=True, stop=True)
                nc.vector.tensor_copy(logits_sb[:], psl[:])
                nc.sync.dma_start(out=out_logits[:], in_=logits_sb[:])

                # ---- u / v ----
                psu = ps_pool.tile([H1, S * M], F32, tag="sm")
                psv = ps_pool.tile([H1, S * M], F32, tag="sm")
                nc.tensor.matmul(psu[:], w1a, featc[:], start=True, stop=True)
                nc.tensor.matmul(psv[:], w1b, featc[:], start=True, stop=True)
                nc.vector.tensor_copy(u_f32[:], psu[:, 0:3 * M])
                nc.vector.tensor_scalar(v_bf[:], psv[:], bg1, None, op0=OP.add)

            # ================= relation scope =================
            with tc.tile_pool(name="pbig", bufs=2, space="PSUM") as pb_pool:
                for bi, (jl, qb, nq) in enumerate(BLOCKS):
                    hdd = hpool.tile([H1, nq, S * M], BF16, tag="hdd")
                    for ql in range(nq):
                        q = qb + ql
                        ucol = u_f32[:, jl * M + q: jl * M + q + 1]
                        if (ql % 32) in HDD_ACT_Q:
                            nc.scalar.activation(hdd[:, ql, :], v_bf[:],
                                                 AF.Relu, bias=ucol)
                        else:
                            nc.vector.tensor_scalar(hdd[:, ql, :], v_bf[:],
                                                    ucol, 0.0,
                                                    op0=OP.add, op1=OP.max)
                    for duo in range(3):
                        iA, iB = 2 * duo, 2 * duo + 1
                        ps = pb_pool.tile([2 * CO, nq * 64], F32, tag="gps")
                        for qg in range(nq // 8):
                            nc.tensor.matmul(
                                ps[0:CO, qg * 512:(qg + 1) * 512],
                                wg2,
                                hdd[:, qg * 8:(qg + 1) * 8, iA * M:(iA + 1) * M],
                                start=True, stop=True)
                            nc.tensor.matmul(
                                ps[CO:2 * CO, qg * 512:(qg + 1) * 512],
                                wg2,
                                hdd[:, qg * 8:(qg + 1) * 8, iB * M:(iB + 1) * M],
                                start=True, stop=True,
                                tile_position=(0, 64))
                        col = 3 * bi + duo
                        gscr = spool.tile([2 * CO, nq * 64], BF16, tag="gscr")
                        nc.scalar.activation(
                            gscr[:], ps[:], AF.Relu, bias=bg2,
                            accum_out=xf_cols[:, col:col + 1])

            nc.sync.dma_start(out=out_xf[:], in_=xf_cols[:])
    nc.compile()
    return nc


_NC_CACHE = None


def _get_nc():
    global _NC_CACHE
    if _NC_CACHE is None:
        _NC_CACHE = _build_nc()
    return _NC_CACHE


def _host_prep(inputs):
    ins = {k: np.asarray(v) for k, v in inputs.items()}
    x = np.concatenate([ins['support_x'], ins['query_x']], axis=1)
    lab = np.concatenate([ins['support_y'], ins['query_y']], axis=1)

    xpad = np.pad(x.astype(np.float32), ((0, 0), (0, 0), (0, 0), (0, 1), (0, 1)))
    win = np.lib.stride_tricks.sliding_window_view(xpad, (3, 3), axis=(3, 4))
    win = win[:, :, :, ::2, ::2]
    patches = win.transpose(0, 2, 5, 6, 1, 3, 4).reshape(B, 27, S, 1024)
    patches = np.ascontiguousarray(patches, np.float32)

    f32 = np.float32
    bf16 = ml_dtypes.bfloat16

    cbf = np.zeros((128, NBF), f32)
    cbf[0:27, O_W1:O_W1 + 32] = ins['k1'].reshape(32, 27).T
    cbf[0:96, O_W2:O_W2 + 144] = ins['k2'].transpose(2, 1, 3, 0).reshape(96, 144)
    cbf[0:48, O_W3:O_W3 + 576] = ins['k3'].transpose(1, 2, 3, 0).reshape(48, 576)
    Wg1 = ins['Wg1'].astype(f32)
    cbf[0:C2, O_W1A:O_W1A + 128] = Wg1[:C2]
    cbf[0:C2, O_W1B:O_W1B + 128] = Wg1[C2:]
    cbf[0:H1, O_WG2:O_WG2 + 64] = ins['Wg2']
    cbf = cbf.astype(bf16)

    cff = np.zeros((128, NF), f32)
    cff[0:32, OF_BC1] = ins['bc1']
    cff[0:48, OF_BC2] = ins['bc2']
    cff[0:64, OF_BC3] = ins['bc3']
    cff[0:H1, OF_BG1] = ins['bg1']
    cff[0:128, OF_BG2] = np.tile(ins['bg2'].astype(f32), 2)
    cff[0:65, OF_WLE:OF_WLE + 64] = np.vstack(
        [ins['Wlog'].astype(f32) / M, ins['blog'][None, :].astype(f32)])
    cff[0:128, OF_BC1Q] = np.tile(ins['bc1'].astype(f32), 4)

    ii = np.arange(D, dtype=f32) / D
    coord = np.stack([np.broadcast_to(ii[:, None], (D, D)),
                      np.broadcast_to(ii[None, :], (D, D))]).reshape(2, M)
    coords = np.ascontiguousarray(np.tile(coord, (1, S)), f32).astype(bf16)

    common = dict(cb=cbf, cf=np.ascontiguousarray(cff), coords=coords)
    in_maps = []
    for core in range(N_CORES):
        b, half = core // 2, core % 2
        # odd cores see images in rotated order so the program's local
        # j in {0,1,2} maps to global j in {3,4,5}
        perm = (0, 1, 2, 3, 4, 5) if half == 0 else (3, 4, 5, 0, 1, 2)
        m = dict(common)
        m['patches'] = np.ascontiguousarray(
            patches[b][:, perm, :]).reshape(27, S * 1024).astype(bf16)
        in_maps.append(m)
    return in_maps, lab, ins


def _host_post(results, lab, ins):
    f32 = np.float32
    # ---- cls loss from raw logits (even cores have identity perm) ----
    cls_terms = np.zeros((B, S), f32)
    for b in range(B):
        logits = results[2 * b]["logits"].astype(f32)          # [6, 64]
        mx = logits.max(axis=1, keepdims=True)
        lse = np.log(np.exp(logits - mx).sum(axis=1, keepdims=True)) + mx
        logp = logits - lse
        cls_terms[b] = lse[:, 0] - logits[np.arange(S), lab[b]]
    cls_loss = np.float32(cls_terms.mean())

    # ---- score head from relation sums ----
    xf = np.zeros((B, S, S, 2 * CO), f32)   # [b, i_loc?, ...]
    P = np.zeros((B, S, S), f32)
    Wf1, bf1 = ins['Wf1'].astype(f32), ins['bf1'].astype(f32)
    Wf2, bf2 = ins['Wf2'].astype(f32), ins['bf2'].astype(f32)
    for core in range(N_CORES):
        b, half = core // 2, core % 2
        perm = (0, 1, 2, 3, 4, 5) if half == 0 else (3, 4, 5, 0, 1, 2)
        dev = results[core]["xf"].astype(f32)                  # [128, NXF]
        for jl in range(3):
            for duo in range(3):
                colsum = np.zeros(128, f32)
                for bi, (bjl, _, _) in enumerate(BLOCKS):
                    if bjl == jl:
                        colsum += dev[:, 3 * bi + duo]
                for ih in range(2):
                    i_loc = 2 * duo + ih
                    x_f = colsum[ih * CO:(ih + 1) * CO]
                    h = np.maximum(x_f @ Wf1 + bf1, 0.0)
                    s2 = h @ Wf2 + bf2
                    P[b, perm[i_loc], perm[jl]] = 1.0 / (1.0 + np.exp(-s2[0]))

    y = (lab[:, :, None] == lab[:, None, :]).astype(f32)
    Pt = P.transpose(0, 2, 1)
    sym, anti = f32(0.5) * (P + Pt), f32(0.5) * (P - Pt)
    sym_n = np.sqrt((sym ** 2).sum(axis=(1, 2)))
    anti_n = np.sqrt((anti ** 2).sum(axis=(1, 2)))
    sym_loss = np.float32(((sym_n - anti_n) / (sym_n + anti_n)).mean())
    euc_loss = np.float32(((P - y) ** 2).mean())
    rn_loss = np.float32(euc_loss - np.float32(0.1) * sym_loss)
    return np.float32(cls_loss), np.float32(rn_loss), np.float32(sym_loss)


def run_spmd(inputs, trace=False, **kwargs):
    nc = _get_nc()
    in_maps, lab, ins = _host_prep(inputs)
    res = run_bass_kernel_spmd(nc, in_maps, list(range(N_CORES)),
                               trace=trace, **kwargs)
    return _host_post(res.results, lab, ins), res


def kernel(**inputs):
    out, _ = run_spmd(inputs)
    return out
